# revision 1
# baseline (speedup 1.0000x reference)
"""ABCNN-3 block on 8 TRN2 NeuronCores, data-parallel over batch.

Full inputs in, full outputs out. Internally: batch 64 -> 8 cores x 8.
Each core runs the whole pipeline for its 8 batch elements:
  F = Fr * mask
  A = 1/(1 + pdist(F0, F1))           (= Sigmoid(-0.5 * Ln(sq)))
  F0a = A^T @ W0 ; F1a = A @ W1
  out0 = tanh(conv_{w}([F0; F0a]) + b); out1 likewise
  A2 = 1/(1 + pdist(out0, out1)); a0 = rowsum, a1 = colsum
  o0 = avgpool4(out0 * a0); o1 = avgpool4(out1 * a1)

Layouts: "natural" = [row-part, col-free]; "T" = transposed [d-part, t-free].
The width-4 conv along t runs on TensorE as a banded matmul with a shared
[128, 131] band block; Ln/Sigmoid fuse the sqrt+reciprocal; norms ride the
matmul K-rows / activation bias; broadcasts across partitions are K=1 matmuls.
"""

import os
import sys

import numpy as np

for _p in ("/opt/trn_rl_repo",):
    if _p not in sys.path and os.path.isdir(_p):
        sys.path.insert(0, _p)

import concourse.bacc as bacc
import concourse.bass as bass
import concourse.mybir as mybir
import concourse.tile as tile
from concourse import masks
from concourse.bass_utils import run_bass_kernel_spmd

F32 = mybir.dt.float32
BF16 = mybir.dt.bfloat16
AF = mybir.ActivationFunctionType
ALU = mybir.AluOpType

B, L, D, W = 64, 512, 512, 4
NCORES = 8
BPC = B // NCORES          # batches per core
LP = L + W - 1             # 515
P = 128
NC = L // P                # 4 chunks of 128 over both L and D
NT = (LP + P - 1) // P     # 5 chunks over LP (last has 3 rows)


def _build(dbg=False):
    nc = bacc.Bacc("TRN2", target_bir_lowering=False, debug=False,
                   num_devices=NCORES)

    f0r = nc.declare_dram_parameter("F0r", [BPC, L, D], F32, isOutput=False)
    f1r = nc.declare_dram_parameter("F1r", [BPC, L, D], F32, isOutput=False)
    m0d = nc.declare_dram_parameter("sent0_mask", [BPC, L], F32, isOutput=False)
    m1d = nc.declare_dram_parameter("sent1_mask", [BPC, L], F32, isOutput=False)
    w0d = nc.declare_dram_parameter("W0", [L, D], F32, isOutput=False)
    w1d = nc.declare_dram_parameter("W1", [L, D], F32, isOutput=False)
    cwd = nc.declare_dram_parameter("conv_w", [1, 2, W, 1], F32, isOutput=False)
    cbd = nc.declare_dram_parameter("conv_b", [1], F32, isOutput=False)
    outd = nc.declare_dram_parameter("out", [2, BPC, L, D], F32, isOutput=True)
    if dbg:
        dA = nc.declare_dram_parameter("dA", [L, L], F32, isOutput=True)
        dF0T = nc.declare_dram_parameter("dF0T", [D, L], F32, isOutput=True)
        dAC1 = nc.declare_dram_parameter("dAC1", [L, LP], F32, isOutput=True)
        dATC1 = nc.declare_dram_parameter("dATC1", [L, LP], F32, isOutput=True)
        dO0T = nc.declare_dram_parameter("dO0T", [D, LP], F32, isOutput=True)
        dO1T = nc.declare_dram_parameter("dO1T", [D, LP], F32, isOutput=True)
        dROW = nc.declare_dram_parameter("dROW", [8, LP], F32, isOutput=True)
        dN = nc.declare_dram_parameter("dN", [P, 8], F32, isOutput=True)

    with tile.TileContext(nc) as tc:
        with (
            tc.tile_pool(name="const", bufs=1) as cpool,
            tc.tile_pool(name="work", bufs=2) as wp,
            tc.tile_pool(name="pt", bufs=2, space="PSUM") as pt,
            tc.tile_pool(name="pm", bufs=2, space="PSUM") as pm,
        ):
            # ---------------- constants ----------------
            ident_bf = cpool.tile([P, P], BF16)
            masks.make_identity(nc, ident_bf[:])
            ident_f = cpool.tile([P, P], F32)
            masks.make_identity(nc, ident_f[:])
            ones_row = cpool.tile([1, P], BF16)
            nc.gpsimd.memset(ones_row[:], 1.0)
            ones_col = cpool.tile([P, 1], BF16)
            nc.gpsimd.memset(ones_col[:], 1.0)
            ones_f11 = cpool.tile([1, 1], F32)
            nc.gpsimd.memset(ones_f11[:], 1.0)

            # conv params
            cw_sb = wp.tile([1, 2 * W], F32)
            nc.sync.dma_start(cw_sb[:], cwd.rearrange("a b c d -> a (b c d)"))
            cw_bf = wp.tile([1, 2 * W], BF16)
            nc.vector.tensor_copy(cw_bf[:], cw_sb[:])
            cb_sb = wp.tile([1, 1], F32)
            nc.sync.dma_start(cb_sb[:], cbd[None, :])
            cb_bf = wp.tile([1, 1], BF16)
            nc.vector.tensor_copy(cb_bf[:], cb_sb[:])

            # broadcast conv coeffs + bias across partitions (K=1 matmuls)
            pmc = pm.tile([P, 520], F32, tag="pm")
            nc.tensor.matmul(pmc[:, 0:2 * W], ones_row[:], cw_bf[:])
            nc.tensor.matmul(pmc[:, 2 * W:2 * W + 1], ones_row[:], cb_bf[:])
            ccols = cpool.tile([P, 2 * W], F32)
            nc.scalar.copy(ccols[:], pmc[:, 0:2 * W])
            cb_col = cpool.tile([P, 1], F32)
            nc.scalar.copy(cb_col[:], pmc[:, 2 * W:2 * W + 1])

            # band blocks: band_ch[p, q] = conv_w[ch][3 - (q-p)] for q-p in [0,4)
            bands = []
            for ch in range(2):
                band = cpool.tile([P, P + W - 1], BF16, tag=f"band{ch}")
                nc.gpsimd.memset(band[:], 0.0)
                for r in range(W):
                    msk = wp.tile([P, P + W - 1], BF16, tag="bandmask")
                    nc.gpsimd.memset(msk[:], 0.0)
                    nc.gpsimd.affine_select(
                        out=msk[:], in_=msk[:],
                        compare_op=ALU.not_equal, fill=1.0,
                        base=r, pattern=[[-1, P + W - 1]], channel_multiplier=1,
                    )
                    # band += msk * c[ch, 3-r]
                    nc.vector.scalar_tensor_tensor(
                        out=band[:], in0=msk[:],
                        scalar=ccols[:, 4 * ch + (W - 1 - r):4 * ch + (W - r)],
                        in1=band[:], op0=ALU.mult, op1=ALU.add,
                    )
                bands.append(band)
            band0, band1 = bands

            # W0 / W1 in bf16, natural layout [i_p, chunk, d]
            wbfs = []
            for name, wd in (("w0", w0d), ("w1", w1d)):
                wbf = cpool.tile([P, NC, D], BF16, tag=f"{name}bf")
                wdr = wd.rearrange("(c p) d -> c p d", p=P)
                for c in range(NC):
                    wtmp = wp.tile([P, D], F32, tag="wtmp")
                    nc.sync.dma_start(wtmp[:], wdr[c])
                    nc.vector.tensor_copy(wbf[:, c, :], wtmp[:])
                wbfs.append(wbf)
            w0bf, w1bf = wbfs

            def dump_tile(dst, tl, nch, x):
                for c in range(nch):
                    sc = wp.tile([P, x], F32, tag="dbgsc")
                    nc.vector.tensor_copy(sc[:], tl[:, c, :])
                    nc.sync.dma_start(dst[P * c:P * (c + 1), :], sc[:])

            def dump_row(dst_row, row_ap, x):
                sc = wp.tile([1, 520], F32, tag="dbgrow")
                nc.vector.tensor_copy(sc[0:1, 0:x], row_ap)
                nc.sync.dma_start(dst_row, sc[0:1, 0:x])

            # ---------------- per-batch pipeline ----------------
            for b in range(BPC):
                # --- load + mask + cast ---
                f0f = wp.tile([P, NC, D], F32, tag="f0f")
                f1f = wp.tile([P, NC, D], F32, tag="f1f")
                nc.sync.dma_start(f0f[:], f0r[b].rearrange("(c p) d -> p c d", p=P))
                nc.sync.dma_start(f1f[:], f1r[b].rearrange("(c p) d -> p c d", p=P))
                m0 = wp.tile([P, NC], F32, tag="m0")
                m1 = wp.tile([P, NC], F32, tag="m1")
                nc.sync.dma_start(m0[:], m0d[b].rearrange("(c p) -> p c", p=P))
                nc.sync.dma_start(m1[:], m1d[b].rearrange("(c p) -> p c", p=P))

                f0n = wp.tile([P, NC, D], BF16, tag="f0n")
                f1n = wp.tile([P, NC, D], BF16, tag="f1n")
                for c in range(NC):
                    nc.vector.tensor_scalar_mul(f0n[:, c, :], f0f[:, c, :],
                                                m0[:, c:c + 1])
                    nc.vector.tensor_scalar_mul(f1n[:, c, :], f1f[:, c, :],
                                                m1[:, c:c + 1])

                # --- norms (ACT square w/ accumulate) ---
                n0c = wp.tile([P, NC], F32, tag="n0c")
                n1c = wp.tile([P, NC], F32, tag="n1c")
                for c in range(NC):
                    sqs = wp.tile([P, D], BF16, tag="sqs")
                    nc.scalar.activation(sqs[:], f0n[:, c, :], AF.Square,
                                         accum_out=n0c[:, c:c + 1])
                    sqs2 = wp.tile([P, D], BF16, tag="sqs2")
                    nc.scalar.activation(sqs2[:], f1n[:, c, :], AF.Square,
                                         accum_out=n1c[:, c:c + 1])

                # n1 as a row, scaled by -0.5 (for the G matmul K-row)
                pr = pm.tile([P, 520], F32, tag="pm")
                for c in range(NC):
                    nc.tensor.matmul(pr[0:1, P * c:P * (c + 1)], n1c[:, c:c + 1],
                                     ident_f[:])
                n1row = wp.tile([1, L], BF16, tag="n1row")
                nc.scalar.activation(n1row[:], pr[0:1, 0:L], AF.Copy, scale=-0.5)

                # --- transpose F0, F1 to [d_p, c_d, i] ---
                f0T = wp.tile([P, NC, L], BF16, tag="f0T")
                f1T = wp.tile([P, NC, L], BF16, tag="f1T")
                for src, dst in ((f0n, f0T), (f1n, f1T)):
                    for cd in range(NC):
                        ptt = pt.tile([P, 512], BF16, tag="ptt")
                        for ci in range(NC):
                            nc.tensor.transpose(
                                ptt[:, P * ci:P * (ci + 1)],
                                src[:, ci, P * cd:P * (cd + 1)], ident_bf[:])
                        nc.vector.tensor_copy(dst[:, cd, :], ptt[:])

                # --- G = F0 @ F1^T (minus n1/2 row); A = sigmoid(-0.5 ln sq) ---
                A_ = wp.tile([P, NC, L], BF16, tag="A_")
                for ci in range(NC):
                    pg = pm.tile([P, 520], F32, tag="pm")
                    for cd in range(NC):
                        nc.tensor.matmul(pg[:, 0:L],
                                         f0T[:, cd, P * ci:P * (ci + 1)],
                                         f1T[:, cd, :],
                                         start=(cd == 0), stop=False)
                    nc.tensor.matmul(pg[:, 0:L], ones_row[:], n1row[:],
                                     start=False, stop=True)
                    lnt = wp.tile([P, L], F32, tag="lnt")
                    nc.scalar.activation(lnt[:], pg[:, 0:L], AF.Ln,
                                         bias=n0c[:, ci:ci + 1], scale=-2.0)
                    nc.scalar.activation(A_[:, ci, :], lnt[:], AF.Sigmoid,
                                         scale=-0.5)

                if dbg and b == 0:
                    dump_tile(dA, A_, NC, L)
                    dump_tile(dF0T, f0T, NC, L)
                    nc.sync.dma_start(dN[:, 0:NC], n0c[:])
                    nc.sync.dma_start(dN[:, NC:2 * NC], n1c[:])

                # --- A^T ---
                AT_ = wp.tile([P, NC, L], BF16, tag="AT_")
                for cj in range(NC):
                    ptt = pt.tile([P, 512], BF16, tag="ptt")
                    for ci in range(NC):
                        nc.tensor.transpose(ptt[:, P * ci:P * (ci + 1)],
                                            A_[:, ci, P * cj:P * (cj + 1)],
                                            ident_bf[:])
                    nc.vector.tensor_copy(AT_[:, cj, :], ptt[:])

                # --- banded convs of A / A^T along free axis (channel-1) ---
                # Ac1[i, t] = sum_j A[i, j] band1[j, t] ; lhsT = AT_
                # ATc1[j, t] = sum_i A^T[j, i] band1[i, t] ; lhsT = A_
                def band_chain(pb, lhs_tile, co, bandt, starts=True):
                    # interleaved main/tail so each accumulate follows the
                    # start=True that owns its column region
                    for ck in range(NC):
                        nc.tensor.matmul(
                            pb[:, P * ck:P * (ck + 1)],
                            lhs_tile[:, ck, P * co:P * (co + 1)],
                            bandt[:, 0:P],
                            start=starts, stop=False, skip_group_check=True)
                        if ck > 0:
                            nc.tensor.matmul(
                                pb[:, P * ck:P * ck + W - 1],
                                lhs_tile[:, ck - 1, P * co:P * (co + 1)],
                                bandt[:, P:P + W - 1],
                                start=False, stop=False, skip_group_check=True)
                    nc.tensor.matmul(
                        pb[:, P * NC:P * NC + W - 1],
                        lhs_tile[:, NC - 1, P * co:P * (co + 1)],
                        bandt[:, P:P + W - 1],
                        start=starts, stop=False, skip_group_check=True)

                def band_conv(lhs_tile, out_tag):
                    outs = wp.tile([P, NC, LP], BF16, tag=out_tag)
                    for co in range(NC):  # output row chunk
                        pb = pm.tile([P, 520], F32, tag="pm")
                        band_chain(pb, lhs_tile, co, band1)
                        nc.vector.tensor_copy(outs[:, co, :], pb[:, 0:LP])
                    return outs

                ac1 = band_conv(AT_, "ac1")
                atc1 = band_conv(A_, "atc1")

                if dbg and b == 0:
                    dump_tile(dAC1, ac1, NC, LP)
                    dump_tile(dATC1, atc1, NC, LP)

                # --- conv + tanh -> out0T/out1T in [d_p, c_d, t] ---
                def conv_out(wbf, acts, fn, out_tag):
                    outT = wp.tile([P, NC, LP], BF16, tag=out_tag)
                    for cd in range(NC):
                        po = pm.tile([P, 520], F32, tag="pm")
                        # channel-1 W @ (A*band) starts the full region...
                        for ci in range(NC):
                            nc.tensor.matmul(po[:, 0:512],
                                             wbf[:, ci, P * cd:P * (cd + 1)],
                                             acts[:, ci, 0:512],
                                             start=(ci == 0), stop=False,
                                             skip_group_check=True)
                            nc.tensor.matmul(po[:, 512:LP],
                                             wbf[:, ci, P * cd:P * (cd + 1)],
                                             acts[:, ci, 512:LP],
                                             start=(ci == 0), stop=False,
                                             skip_group_check=True)
                        # ...then the channel-0 banded conv accumulates
                        band_chain(po, fn, cd, band0, starts=False)
                        nc.scalar.activation(outT[:, cd, :], po[:, 0:LP],
                                             AF.Tanh, bias=cb_col[:])
                    return outT

                out0T = conv_out(w0bf, ac1, f0n, "out0T")
                out1T = conv_out(w1bf, atc1, f1n, "out1T")

                # --- norms of out rows: nt[t] = sum_d outT[d,t]^2 (rows) ---
                ntrows = []
                for src, tag in ((out0T, "nt0"), (out1T, "nt1")):
                    pn = pm.tile([P, 520], F32, tag="pm")
                    for cd in range(NC):
                        sqo = wp.tile([P, LP], BF16, tag="sqo")
                        nc.vector.tensor_mul(sqo[:], src[:, cd, :], src[:, cd, :])
                        nc.tensor.matmul(pn[0:1, 0:512], ones_col[:],
                                         sqo[:, 0:512],
                                         start=(cd == 0), stop=False)
                        nc.tensor.matmul(pn[0:1, 512:LP], ones_col[:],
                                         sqo[:, 512:LP],
                                         start=(cd == 0), stop=(cd == NC - 1))
                    row = wp.tile([1, LP], F32, tag=f"{tag}row")
                    nc.scalar.copy(row[:], pn[0:1, 0:LP])
                    ntrows.append(row)
                nt0row, nt1row = ntrows

                if dbg and b == 0:
                    dump_tile(dO0T, out0T, NC, LP)
                    dump_tile(dO1T, out1T, NC, LP)
                    dump_row(dROW[0:1, :], nt0row[:], LP)
                    dump_row(dROW[1:2, :], nt1row[:], LP)

                # nt1 as bf16 row scaled by -0.5 (K-row of G2)
                nt1row_s = wp.tile([1, LP], BF16, tag="nt1row_s")
                nc.scalar.activation(nt1row_s[:], nt1row[:], AF.Copy, scale=-0.5)
                # nt0 as per-partition columns (bias of the Ln pass)
                pcn = pm.tile([P, 520], F32, tag="pm")
                for ct in range(NT):
                    M = min(P, LP - P * ct)
                    nc.tensor.matmul(pcn[0:M, ct:ct + 1],
                                     nt0row[0:1, P * ct:P * ct + M], ones_f11[:])
                nt0col = wp.tile([P, NT], F32, tag="nt0col")
                nc.scalar.copy(nt0col[:], pcn[:, 0:NT])

                # --- G2 + A2 ; a0 = rowsum (accum), a1 = colsum (ones matmul) ---
                a2s = wp.tile([P, NT, LP], BF16, tag="a2s")
                a0jc = wp.tile([P, NT], F32, tag="a0jc")
                for ct in range(NT):
                    M = min(P, LP - P * ct)
                    pg2 = pm.tile([P, 520], F32, tag="pm")
                    for cd in range(NC):
                        nc.tensor.matmul(pg2[0:M, 0:512],
                                         out0T[:, cd, P * ct:P * ct + M],
                                         out1T[:, cd, 0:512],
                                         start=(cd == 0), stop=False)
                        nc.tensor.matmul(pg2[0:M, 512:LP],
                                         out0T[:, cd, P * ct:P * ct + M],
                                         out1T[:, cd, 512:LP],
                                         start=(cd == 0), stop=False)
                    nc.tensor.matmul(pg2[0:M, 0:512], ones_row[0:1, 0:M],
                                     nt1row_s[0:1, 0:512], start=False, stop=False)
                    nc.tensor.matmul(pg2[0:M, 512:LP], ones_row[0:1, 0:M],
                                     nt1row_s[0:1, 512:LP], start=False, stop=True)
                    lnt2 = wp.tile([P, LP], F32, tag="lnt2")
                    nc.scalar.activation(lnt2[0:M, :], pg2[0:M, 0:LP], AF.Ln,
                                         bias=nt0col[0:M, ct:ct + 1], scale=-2.0)
                    nc.scalar.activation(a2s[0:M, ct, :], lnt2[0:M, :],
                                         AF.Sigmoid, scale=-0.5,
                                         accum_out=a0jc[0:M, ct:ct + 1])

                # a1: column sums via ones-matmuls over all row chunks
                pa1 = pm.tile([P, 520], F32, tag="pm")
                for ct in range(NT):
                    M = min(P, LP - P * ct)
                    nc.tensor.matmul(pa1[0:1, 0:512], ones_col[0:M, 0:1],
                                     a2s[0:M, ct, 0:512],
                                     start=(ct == 0), stop=False)
                    nc.tensor.matmul(pa1[0:1, 512:LP], ones_col[0:M, 0:1],
                                     a2s[0:M, ct, 512:LP],
                                     start=(ct == 0), stop=(ct == NT - 1))

                # --- broadcast 0.25*a0 / 0.25*a1 across partitions ---
                # a0: cols -> row
                pra = pm.tile([P, 520], F32, tag="pm")
                for ct in range(NT):
                    M = min(P, LP - P * ct)
                    nc.tensor.matmul(pra[0:1, P * ct:P * ct + M],
                                     a0jc[0:M, ct:ct + 1], ident_f[0:M, 0:M],
                                     start=True, stop=True)
                a0row = wp.tile([1, LP], BF16, tag="a0row")
                nc.scalar.activation(a0row[:], pra[0:1, 0:LP], AF.Copy, scale=0.25)
                a1row = wp.tile([1, LP], BF16, tag="a1row")
                nc.scalar.activation(a1row[:], pa1[0:1, 0:LP], AF.Copy, scale=0.25)

                if dbg and b == 0:
                    dump_row(dROW[2:3, :], a0row[:], LP)
                    dump_row(dROW[3:4, :], a1row[:], LP)

                bcs = []
                for row, tag in ((a0row, "bc0"), (a1row, "bc1")):
                    pbc = pm.tile([P, 520], F32, tag="pm")
                    nc.tensor.matmul(pbc[:, 0:512], ones_row[:], row[0:1, 0:512],
                                     start=True, stop=False)
                    nc.tensor.matmul(pbc[:, 512:LP], ones_row[:],
                                     row[0:1, 512:LP], start=True, stop=True)
                    bc = wp.tile([P, LP], BF16, tag=tag)
                    nc.vector.tensor_copy(bc[:], pbc[:, 0:LP])
                    bcs.append(bc)
                bc0, bc1 = bcs

                # --- scale, pool, transpose back, store ---
                for oi, (srcT, bc) in enumerate(((out0T, bc0), (out1T, bc1))):
                    op_ = wp.tile([P, NC, L], BF16, tag=f"op{oi}")
                    for cd in range(NC):
                        s_ = wp.tile([P, LP], BF16, tag="s_")
                        nc.vector.tensor_mul(s_[:], srcT[:, cd, :], bc[:])
                        u1 = wp.tile([P, LP - 1], BF16, tag="u1")
                        nc.vector.tensor_add(u1[:], s_[:, 0:LP - 1], s_[:, 1:LP])
                        nc.vector.tensor_add(op_[:, cd, :], u1[:, 0:L],
                                             u1[:, 2:L + 2])
                    for ct in range(NC):
                        pto = pt.tile([P, 512], BF16, tag="pto")
                        for cd in range(NC):
                            nc.tensor.transpose(pto[:, P * cd:P * (cd + 1)],
                                                op_[:, cd, P * ct:P * (ct + 1)],
                                                ident_bf[:])
                        onat = wp.tile([P, D], F32, tag="onat")
                        nc.scalar.copy(onat[:], pto[:])
                        nc.sync.dma_start(
                            outd[oi, b, P * ct:P * (ct + 1), :], onat[:])

    nc.compile()
    return nc


_CACHE = {}


def _get_nc(dbg=False):
    key = f"nc{int(dbg)}"
    if key not in _CACHE:
        _CACHE[key] = _build(dbg)
    return _CACHE[key]


def _in_maps(F0r, F1r, sent0_mask, sent1_mask, W0, W1, conv_w, conv_b):
    maps = []
    for c in range(NCORES):
        s = slice(c * BPC, (c + 1) * BPC)
        maps.append({
            "F0r": np.ascontiguousarray(F0r[s], np.float32),
            "F1r": np.ascontiguousarray(F1r[s], np.float32),
            "sent0_mask": np.ascontiguousarray(sent0_mask[s], np.float32),
            "sent1_mask": np.ascontiguousarray(sent1_mask[s], np.float32),
            "W0": np.ascontiguousarray(W0, np.float32),
            "W1": np.ascontiguousarray(W1, np.float32),
            "conv_w": np.ascontiguousarray(conv_w, np.float32),
            "conv_b": np.ascontiguousarray(conv_b, np.float32),
        })
    return maps


def run(trace=False, dbg=False, **inputs):
    nc = _get_nc(dbg)
    res = run_bass_kernel_spmd(nc, _in_maps(**inputs),
                               core_ids=list(range(NCORES)), trace=trace)
    o0 = np.concatenate([r["out"][0] for r in res.results], axis=0)
    o1 = np.concatenate([r["out"][1] for r in res.results], axis=0)
    return (o0, o1), res


def kernel(**inputs):
    outs, _ = run(trace=False, **inputs)
    return outs



# revision 22
# speedup vs baseline: 1.1376x; 1.1376x over previous
"""ABCNN-3 block on 8 TRN2 NeuronCores, data-parallel over batch.

Full inputs in, full outputs out. Internally: batch 64 -> 8 cores x 8.
Each core runs the whole pipeline for its 8 batch elements:
  F = Fr * mask
  A = 1/(1 + pdist(F0, F1))
  F0a = A^T @ W0 ; F1a = A @ W1
  out0 = tanh(conv_w([F0; F0a]) + b); out1 likewise
  A2 = 1/(1 + pdist(out0, out1)); a0 = rowsum, a1 = colsum
  o0 = avgpool4(out0 * a0); o1 = avgpool4(out1 * a1)

v2 design notes (vs the v1 baseline):
  - Scalar engine runs ONLY Sqrt + Tanh (2 act tables, ~2 loads/batch).
    A = 1/(1+s) finishes on DVE (add + reciprocal in bf16).
  - All transposes (F0, F1, A) via DMA XBAR (dma_start_transpose), no PE
    transposes and no PSUM->SBUF copies for them.
  - Band convs: gpsimd memset of the PSUM region + 131-col windowed
    matmuls (no tiny tail matmuls, no start-group juggling).
  - Norms via DVE tensor_tensor_reduce (no Square activations).
  - Output written in [d, t] layout straight from the pooling adds
    (fp32), transposed on the host. No PE output transposes, no scalar
    copies.
  - Two-stage software pipeline (H1: load..A^T, H2: bands..store),
    emitted with a one-batch skew so the PE stays busy during the
    sqrt/reciprocal/transpose chain.
"""

import os
import sys

import numpy as np

for _p in ("/opt/trn_rl_repo",):
    if _p not in sys.path and os.path.isdir(_p):
        sys.path.insert(0, _p)

import concourse.bacc as bacc
import concourse.bass as bass
import concourse.mybir as mybir
import concourse.tile as tile
from concourse import masks
from concourse.bass_utils import run_bass_kernel_spmd

F32 = mybir.dt.float32
BF16 = mybir.dt.bfloat16
AF = mybir.ActivationFunctionType
ALU = mybir.AluOpType
AX = mybir.AxisListType

B, L, D, W = 64, 512, 512, 4
NCORES = 8
BPC = B // NCORES          # batches per core
LP = L + W - 1             # 515
LPAD = 520
P = 128
NC = L // P                # 4 chunks of 128 over both L and D
NT = (LP + P - 1) // P     # 5 chunks over LP (last has 3 rows)


def _build(bpc=BPC, stop=0):
    # stop: 0 = full kernel; 1..4 = debug builds that end the batch early
    # and dump intermediates into `out` (used only for HW bisection).
    nc = bacc.Bacc("TRN2", target_bir_lowering=False, debug=False,
                   num_devices=NCORES)

    f0r = nc.declare_dram_parameter("F0r", [bpc, L, D], F32, isOutput=False)
    f1r = nc.declare_dram_parameter("F1r", [bpc, L, D], F32, isOutput=False)
    m0d = nc.declare_dram_parameter("sent0_mask", [bpc, L], F32, isOutput=False)
    m1d = nc.declare_dram_parameter("sent1_mask", [bpc, L], F32, isOutput=False)
    w0d = nc.declare_dram_parameter("W0", [L, D], F32, isOutput=False)
    w1d = nc.declare_dram_parameter("W1", [L, D], F32, isOutput=False)
    cwd = nc.declare_dram_parameter("conv_w", [1, 2, W, 1], F32, isOutput=False)
    cbd = nc.declare_dram_parameter("conv_b", [1], F32, isOutput=False)
    # T-layout output [oi, b, d, t]; host transposes to [b, t, d].
    outd = nc.declare_dram_parameter("out", [2, bpc, D, L], F32, isOutput=True)

    with tile.TileContext(nc) as tc:
        with (
            tc.tile_pool(name="const", bufs=1) as cp,
            tc.tile_pool(name="io", bufs=4) as io,
            tc.tile_pool(name="work", bufs=2) as wp,
            tc.tile_pool(name="rows", bufs=2) as rp,
            tc.tile_pool(name="p1", bufs=2, space="PSUM") as p1,
            tc.tile_pool(name="pb", bufs=3, space="PSUM") as pb,
        ):
            # ---------------- constants ----------------
            ident_bf = cp.tile([P, P], BF16)
            masks.make_identity(nc, ident_bf[:])
            ones_row = cp.tile([1, P], BF16)
            nc.gpsimd.memset(ones_row[:], 1.0)
            ones_col = cp.tile([P, 1], BF16)
            nc.gpsimd.memset(ones_col[:], 1.0)
            ones_f11 = cp.tile([1, 1], F32)
            nc.gpsimd.memset(ones_f11[:], 1.0)
            zero_row = cp.tile([1, P], BF16)
            nc.gpsimd.memset(zero_row[:], 0.0)
            zrow512 = cp.tile([1, L], BF16)
            nc.gpsimd.memset(zrow512[:], 0.0)

            # conv params
            cw_sb = rp.tile([1, 2 * W], F32, tag="cw")
            nc.sync.dma_start(cw_sb[:], cwd.rearrange("a b c d -> a (b c d)"))
            cw_bf = rp.tile([1, 2 * W], BF16, tag="cwb")
            nc.vector.tensor_copy(cw_bf[:], cw_sb[:])
            cb_sb = rp.tile([1, 1], F32, tag="cb")
            nc.sync.dma_start(cb_sb[:], cbd[None, :])
            cb_bf = rp.tile([1, 1], BF16, tag="cbb")
            nc.vector.tensor_copy(cb_bf[:], cb_sb[:])

            # broadcast conv coeffs + bias across partitions (K=1 matmuls)
            pmc = p1.tile([P, L], F32, tag="sm")
            nc.tensor.matmul(pmc[:, 0:2 * W], ones_row[:], cw_bf[:])
            nc.tensor.matmul(pmc[:, 2 * W:2 * W + 1], ones_row[:], cb_bf[:])
            ccols = cp.tile([P, 2 * W], F32)
            nc.vector.tensor_copy(ccols[:], pmc[:, 0:2 * W])
            cb_col = cp.tile([P, 1], F32)
            nc.vector.tensor_copy(cb_col[:], pmc[:, 2 * W:2 * W + 1])

            # band blocks: band_ch[q, w] = conv_w[ch][3 - (w-q)] for w-q in [0,4)
            bands = []
            for ch in range(2):
                band = cp.tile([P, P + W - 1], BF16, tag=f"band{ch}")
                nc.gpsimd.memset(band[:], 0.0)
                for r in range(W):
                    msk = rp.tile([P, P + W - 1], BF16, tag="bandmask")
                    nc.gpsimd.memset(msk[:], 0.0)
                    nc.gpsimd.affine_select(
                        out=msk[:], in_=msk[:],
                        compare_op=ALU.not_equal, fill=1.0,
                        base=r, pattern=[[-1, P + W - 1]], channel_multiplier=1,
                    )
                    nc.vector.scalar_tensor_tensor(
                        out=band[:], in0=msk[:],
                        scalar=ccols[:, 4 * ch + (W - 1 - r):4 * ch + (W - r)],
                        in1=band[:], op0=ALU.mult, op1=ALU.add,
                    )
                bands.append(band)
            band0, band1 = bands

            # W0 / W1 in bf16, natural layout [i_p, chunk, d]
            wbfs = []
            for name, wd in (("w0", w0d), ("w1", w1d)):
                wbf = cp.tile([P, NC, D], BF16, tag=f"{name}bf")
                wdr = wd.rearrange("(c p) d -> c p d", p=P)
                for c in range(NC):
                    wtmp = io.tile([P, D], F32, tag="stg")
                    nc.sync.dma_start(wtmp[:], wdr[c])
                    nc.vector.tensor_copy(wbf[:, c, :], wtmp[:])
                wbfs.append(wbf)
            w0bf, w1bf = wbfs

            # ---------------- per-batch stages ----------------
            def stage1(b):
                """load + mask-cast + norms + transposes + G + sqrt + A + A^T"""
                st = {}
                m0 = rp.tile([P, NC], F32, tag="m0")
                m1 = rp.tile([P, NC], F32, tag="m1")
                nc.sync.dma_start(m0[:], m0d[b].rearrange("(c p) -> p c", p=P))
                nc.sync.dma_start(m1[:], m1d[b].rearrange("(c p) -> p c", p=P))

                f0n = wp.tile([P, NC, D], BF16, tag="f0n")
                f1n = wp.tile([P, NC, D], BF16, tag="f1n")
                n0c = rp.tile([P, NC], F32, tag="n0c")
                n1c = rp.tile([P, NC], F32, tag="n1c")
                sqn = wp.tile([P, D], BF16, tag="sqn")
                for src, dst, msk_, ncol in ((f0r, f0n, m0, n0c),
                                             (f1r, f1n, m1, n1c)):
                    srcr = src[b].rearrange("(c p) d -> c p d", p=P)
                    for c in range(NC):
                        stg = io.tile([P, D], F32, tag="stg")
                        nc.sync.dma_start(stg[:], srcr[c])
                        nc.vector.tensor_scalar_mul(dst[:, c, :], stg[:],
                                                    msk_[:, c:c + 1])
                        # norm col = sum_d (mask*F)^2 via Square-accum (the
                        # Square output itself is scratch)
                        nc.scalar.activation(sqn[:], stg[:], AF.Square,
                                             scale=msk_[:, c:c + 1],
                                             accum_out=ncol[:, c:c + 1])

                # DMA XBAR transposes: fT[p, ci, cd, q] = F[128ci+q, 128cd+p]
                f0T = wp.tile([P, NC, NC, P], BF16, tag="f0T")
                f1T = wp.tile([P, NC, NC, P], BF16, tag="f1T")
                nc.sync.dma_start_transpose(f0T[:], f0n[:])
                nc.sync.dma_start_transpose(f1T[:], f1n[:])

                # n1 as a bf16 row scaled by -0.5
                n1b = rp.tile([P, NC], BF16, tag="n1b")
                nc.vector.tensor_copy(n1b[:], n1c[:])
                prr = p1.tile([1, L], F32, tag="sm")
                for c in range(NC):
                    nc.tensor.matmul(prr[0:1, P * c:P * (c + 1)],
                                     n1b[:, c:c + 1], ident_bf[:])
                n1row = rp.tile([1, L], BF16, tag="n1row")
                nc.vector.tensor_scalar_mul(n1row[:], prr[0:1, :], -0.5)

                # G = F0 @ F1^T (minus n1/2 row);
                # A = sigmoid(-0.5 ln(n0 + n1 - 2G)) = 1/(1 + dist).
                # Ln block then Sigmoid block (grouped per act table).
                A_ = wp.tile([P, NC, D], BF16, tag="A_")
                lnts = []
                for ci in range(NC):
                    pg = p1.tile([P, L], F32, tag="sm")
                    for cd in range(NC):
                        nc.tensor.matmul(pg[:, 0:L], f0T[:, ci, cd, :],
                                         f1T[:, :, cd, :],
                                         start=(cd == 0), stop=False)
                    nc.tensor.matmul(pg[:, 0:L], ones_row[:], n1row[0:1, :],
                                     start=False, stop=True)
                    lnt = wp.tile([P, LPAD], F32, tag="lnt", bufs=5)
                    nc.scalar.activation(lnt[:, 0:L], pg[:, 0:L], AF.Ln,
                                         bias=n0c[:, ci:ci + 1], scale=-2.0)
                    lnts.append(lnt)
                for ci in range(NC):
                    nc.scalar.activation(A_[:, ci, :], lnts[ci][:, 0:L],
                                         AF.Sigmoid, scale=-0.5)

                AT_ = wp.tile([P, NC, NC, P], BF16, tag="AT_")
                nc.sync.dma_start_transpose(AT_[:], A_[:])

                st.update(f0n=f0n, f1n=f1n, A_=A_, AT_=AT_)
                return st

            def band_apply(lhs_fn, bandt, dst_tag):
                # dst[m, t] = sum_k lhs[k, m] * band[k, t] over all 4 k-chunks
                dst = wp.tile([P, NC, LPAD], BF16, tag=dst_tag)
                for co in range(NC):
                    pbt = pb.tile([P, LPAD], F32, tag="big")
                    # zero the region with K=1 zero-weight matmuls (GPSIMD
                    # cannot write PSUM on HW)
                    nc.tensor.matmul(pbt[:, 0:L], zero_row[:], zrow512[:],
                                     start=True, stop=False,
                                     skip_group_check=True)
                    nc.tensor.matmul(pbt[:, L:LP], zero_row[:],
                                     zrow512[0:1, 0:LP - L],
                                     start=True, stop=False,
                                     skip_group_check=True)
                    for ck in range(NC - 1):
                        nc.tensor.matmul(
                            pbt[:, P * ck:P * ck + P + W - 1],
                            lhs_fn(co, ck), bandt[:, 0:P + W - 1],
                            start=False, stop=False, skip_group_check=True)
                    nc.tensor.matmul(
                        pbt[:, P * (NC - 1):L],
                        lhs_fn(co, NC - 1), bandt[:, 0:P],
                        start=False, stop=False, skip_group_check=True)
                    nc.tensor.matmul(
                        pbt[:, L:LP],
                        lhs_fn(co, NC - 1), bandt[:, P:P + W - 1],
                        start=False, stop=True, skip_group_check=True)
                    nc.vector.tensor_copy(dst[:, co, 0:LP], pbt[:, 0:LP])
                return dst

            def conv_out(wbf, acts, fn, out_tag):
                outT = wp.tile([P, NC, LPAD], BF16, tag=out_tag)
                for cd in range(NC):
                    pc = pb.tile([P, LPAD], F32, tag="big")
                    for ci in range(NC):
                        wsl = wbf[:, ci, P * cd:P * (cd + 1)]
                        nc.tensor.matmul(pc[:, 0:L], wsl, acts[:, ci, 0:L],
                                         start=(ci == 0), stop=False,
                                         skip_group_check=True)
                        nc.tensor.matmul(pc[:, L:LP], wsl, acts[:, ci, L:LP],
                                         start=(ci == 0), stop=False,
                                         skip_group_check=True)
                    for ck in range(NC - 1):
                        nc.tensor.matmul(
                            pc[:, P * ck:P * ck + P + W - 1],
                            fn[:, ck, P * cd:P * (cd + 1)],
                            band0[:, 0:P + W - 1],
                            start=False, stop=False, skip_group_check=True)
                    nc.tensor.matmul(
                        pc[:, P * (NC - 1):L],
                        fn[:, NC - 1, P * cd:P * (cd + 1)], band0[:, 0:P],
                        start=False, stop=False, skip_group_check=True)
                    nc.tensor.matmul(
                        pc[:, L:LP],
                        fn[:, NC - 1, P * cd:P * (cd + 1)],
                        band0[:, P:P + W - 1],
                        start=False, stop=True, skip_group_check=True)
                    nc.scalar.activation(outT[:, cd, 0:LP], pc[:, 0:LP],
                                         AF.Tanh, bias=cb_col[:])
                return outT

            def stage2(b, st):
                """bands + conv/tanh + out-norms + G2 + A2 + pool + store"""
                A_, AT_ = st["A_"], st["AT_"]
                def dump(oi, c, src_ap):
                    of = io.tile([P, D], F32, tag="of")
                    nc.vector.tensor_copy(of[:], src_ap)
                    nc.sync.dma_start(outd[oi, b, P * c:P * (c + 1), :], of[:])

                if stop == 1:
                    for c in range(NC):
                        dump(0, c, A_[:, c, :])
                        dump(1, c, AT_[:, :, c, :])
                    return
                ac1 = band_apply(lambda co, ck: AT_[:, co, ck, :], band1, "ac1")
                atc1 = band_apply(
                    lambda co, ck: A_[:, ck, P * co:P * (co + 1)], band1,
                    "atc1")

                if stop == 2:
                    for c in range(NC):
                        dump(0, c, ac1[:, c, 0:L])
                        dump(1, c, atc1[:, c, 0:L])
                    return

                out0T = conv_out(w0bf, ac1, st["f0n"], "out0T")
                out1T = conv_out(w1bf, atc1, st["f1n"], "out1T")

                if stop == 3:
                    for c in range(NC):
                        dump(0, c, out0T[:, c, 0:L])
                        dump(1, c, out1T[:, c, 0:L])
                    return

                # norms of out rows: nt[t] = sum_d outT[d, t]^2
                sqo = wp.tile([P, NC, LPAD], BF16, tag="sqo")
                ntpieces = []
                for src in (out0T, out1T):
                    nc.vector.tensor_mul(sqo[:, :, 0:LP], src[:, :, 0:LP],
                                         src[:, :, 0:LP])
                    pn5 = p1.tile([1, L], F32, tag="sm")
                    pn3 = p1.tile([1, 8], F32, tag="sm")
                    for cd in range(NC):
                        nc.tensor.matmul(pn5[0:1, :], ones_col[:],
                                         sqo[:, cd, 0:L],
                                         start=(cd == 0), stop=(cd == NC - 1))
                        nc.tensor.matmul(pn3[0:1, 0:LP - L], ones_col[:],
                                         sqo[:, cd, L:LP],
                                         start=(cd == 0), stop=(cd == NC - 1))
                    ntpieces.append((pn5, pn3))

                # nt0 as f32 row (for the bias cols), nt1 as -0.5-scaled bf16 row
                nt0row = rp.tile([1, LPAD], F32, tag="nt0row")
                nc.vector.tensor_copy(nt0row[0:1, 0:L], ntpieces[0][0][0:1, :])
                nc.vector.tensor_copy(nt0row[0:1, L:LP],
                                      ntpieces[0][1][0:1, 0:LP - L])
                nt1row = rp.tile([1, LPAD], BF16, tag="nt1row")
                nc.vector.tensor_scalar_mul(nt1row[0:1, 0:L],
                                            ntpieces[1][0][0:1, :], -0.5)
                nc.vector.tensor_scalar_mul(nt1row[0:1, L:LP],
                                            ntpieces[1][1][0:1, 0:LP - L], -0.5)

                pcn = p1.tile([P, 8], F32, tag="sm")
                for ct in range(NT):
                    M = min(P, LP - P * ct)
                    nc.tensor.matmul(pcn[0:M, ct:ct + 1],
                                     nt0row[0:1, P * ct:P * ct + M],
                                     ones_f11[:])
                nt0col = rp.tile([P, 8], F32, tag="nt0col")
                nc.vector.tensor_copy(nt0col[:, 0:NC], pcn[:, 0:NC])
                nc.vector.tensor_copy(nt0col[0:LP - L, NC:NT],
                                      pcn[0:LP - L, NC:NT])

                # G2; A2 = sigmoid(-0.5 ln(nt0 + nt1 - 2 G2)); a0 = rowsum
                # rides the Sigmoid accumulator.
                a2s = wp.tile([P, NT, LPAD], BF16, tag="a2s")
                a0jc = rp.tile([P, 8], F32, tag="a0jc")
                lnt2s = []
                for ct in range(NT):
                    M = min(P, LP - P * ct)
                    pq = pb.tile([P, LPAD], F32, tag="big")
                    for cd in range(NC):
                        lsl = out0T[:, cd, P * ct:P * ct + M]
                        nc.tensor.matmul(pq[0:M, 0:L], lsl, out1T[:, cd, 0:L],
                                         start=(cd == 0), stop=False,
                                         skip_group_check=True)
                        nc.tensor.matmul(pq[0:M, L:LP], lsl,
                                         out1T[:, cd, L:LP],
                                         start=(cd == 0), stop=False,
                                         skip_group_check=True)
                    nc.tensor.matmul(pq[0:M, 0:L], ones_row[0:1, 0:M],
                                     nt1row[0:1, 0:L], start=False, stop=False,
                                     skip_group_check=True)
                    nc.tensor.matmul(pq[0:M, L:LP], ones_row[0:1, 0:M],
                                     nt1row[0:1, L:LP], start=False, stop=True,
                                     skip_group_check=True)
                    lnt2 = wp.tile([P, LPAD], F32, tag="lnt", bufs=5)
                    nc.scalar.activation(lnt2[0:M, 0:LP], pq[0:M, 0:LP],
                                         AF.Ln, bias=nt0col[0:M, ct:ct + 1],
                                         scale=-2.0)
                    lnt2s.append((lnt2, M))
                for ct in range(NT):
                    lnt2, M = lnt2s[ct]
                    nc.scalar.activation(a2s[0:M, ct, 0:LP], lnt2[0:M, 0:LP],
                                         AF.Sigmoid, scale=-0.5,
                                         accum_out=a0jc[0:M, ct:ct + 1])

                # a1 = colsum via ones matmuls
                pa5 = p1.tile([1, L], F32, tag="sm")
                pa3 = p1.tile([1, 8], F32, tag="sm")
                for ct in range(NT):
                    M = min(P, LP - P * ct)
                    nc.tensor.matmul(pa5[0:1, :], ones_col[0:M, 0:1],
                                     a2s[0:M, ct, 0:L],
                                     start=(ct == 0), stop=(ct == NT - 1))
                    nc.tensor.matmul(pa3[0:1, 0:LP - L], ones_col[0:M, 0:1],
                                     a2s[0:M, ct, L:LP],
                                     start=(ct == 0), stop=(ct == NT - 1))

                # 0.25-scaled a0/a1 rows, broadcast across partitions
                a0b = rp.tile([P, 8], BF16, tag="a0b")
                nc.vector.tensor_copy(a0b[:, 0:NC], a0jc[:, 0:NC])
                nc.vector.tensor_copy(a0b[0:LP - L, NC:NT],
                                      a0jc[0:LP - L, NC:NT])
                pr5 = p1.tile([1, L], F32, tag="sm")
                pr3 = p1.tile([1, 8], F32, tag="sm")
                for ct in range(NC):
                    nc.tensor.matmul(pr5[0:1, P * ct:P * (ct + 1)],
                                     a0b[:, ct:ct + 1], ident_bf[:])
                nc.tensor.matmul(pr3[0:1, 0:LP - L], a0b[0:LP - L, NT - 1:NT],
                                 ident_bf[0:LP - L, 0:LP - L])
                a0row = rp.tile([1, LPAD], BF16, tag="a0row")
                nc.vector.tensor_scalar_mul(a0row[0:1, 0:L], pr5[0:1, :], 0.25)
                nc.vector.tensor_scalar_mul(a0row[0:1, L:LP],
                                            pr3[0:1, 0:LP - L], 0.25)
                a1row = rp.tile([1, LPAD], BF16, tag="a1row")
                nc.vector.tensor_scalar_mul(a1row[0:1, 0:L], pa5[0:1, :], 0.25)
                nc.vector.tensor_scalar_mul(a1row[0:1, L:LP],
                                            pa3[0:1, 0:LP - L], 0.25)

                bc0 = wp.tile([P, LPAD], BF16, tag="bc0")
                nc.gpsimd.partition_broadcast(bc0[:, 0:LP], a0row[0:1, 0:LP],
                                              channels=P)
                bc1 = wp.tile([P, LPAD], BF16, tag="bc1")
                nc.gpsimd.partition_broadcast(bc1[:, 0:LP], a1row[0:1, 0:LP],
                                              channels=P)

                if stop == 4:
                    dump(0, 0, bc0[:, 0:L])
                    dump(1, 0, bc1[:, 0:L])
                    dump(0, 1, a2s[:, 0, 0:L])
                    return

                # scale + pool4 + store (T layout [d, t], f32)
                for oi, (srcT, bc) in enumerate(((out0T, bc0), (out1T, bc1))):
                    v = wp.tile([P, NC, LPAD], BF16, tag="v")
                    for cd in range(NC):
                        nc.vector.tensor_mul(v[:, cd, 0:LP], srcT[:, cd, 0:LP],
                                             bc[:, 0:LP])
                    u = wp.tile([P, NC, LPAD], BF16, tag="u")
                    nc.vector.tensor_add(u[:, :, 0:LP - 1], v[:, :, 0:LP - 1],
                                         v[:, :, 1:LP])
                    for cd in range(NC):
                        of = io.tile([P, D], F32, tag="of")
                        nc.vector.tensor_add(of[:], u[:, cd, 0:L],
                                             u[:, cd, 2:L + 2])
                        nc.sync.dma_start(outd[oi, b, P * cd:P * (cd + 1), :],
                                          of[:])

            # skewed emission: H1(b+1) before H2(b)
            prev = stage1(0)
            for b in range(bpc):
                nxt = stage1(b + 1) if b + 1 < bpc else None
                stage2(b, prev)
                prev = nxt

    nc.compile()
    return nc


_CACHE = {}


def _get_nc():
    if "nc" not in _CACHE:
        _CACHE["nc"] = _build()
    return _CACHE["nc"]


def _in_maps(F0r, F1r, sent0_mask, sent1_mask, W0, W1, conv_w, conv_b):
    maps = []
    for c in range(NCORES):
        s = slice(c * BPC, (c + 1) * BPC)
        maps.append({
            "F0r": np.ascontiguousarray(F0r[s], np.float32),
            "F1r": np.ascontiguousarray(F1r[s], np.float32),
            "sent0_mask": np.ascontiguousarray(sent0_mask[s], np.float32),
            "sent1_mask": np.ascontiguousarray(sent1_mask[s], np.float32),
            "W0": np.ascontiguousarray(W0, np.float32),
            "W1": np.ascontiguousarray(W1, np.float32),
            "conv_w": np.ascontiguousarray(conv_w, np.float32),
            "conv_b": np.ascontiguousarray(conv_b, np.float32),
        })
    return maps


def run(trace=False, **inputs):
    nc = _get_nc()
    res = run_bass_kernel_spmd(nc, _in_maps(**inputs),
                               core_ids=list(range(NCORES)), trace=trace)
    # out is [2, BPC, D, L] per core; host transposes [d, t] -> [t, d]
    o0 = np.concatenate(
        [r["out"][0].transpose(0, 2, 1) for r in res.results], axis=0)
    o1 = np.concatenate(
        [r["out"][1].transpose(0, 2, 1) for r in res.results], axis=0)
    return (np.ascontiguousarray(o0), np.ascontiguousarray(o1)), res


def kernel(**inputs):
    outs, _ = run(trace=False, **inputs)
    return outs
